# revision 3
# baseline (speedup 1.0000x reference)
"""Faster R-CNN detection head (RPN + NMS + ROI-align + box head) on 8 trn2 cores.

Split:
  device call 1: RPN 3x3 conv + ReLU + 1x1 cls/bbox heads over 5 FPN levels
                 (rows sharded across 8 cores; conv = 9 shifted matmuls
                 accumulated in PSUM).
  host:          per-level top-k, box decode, greedy NMS, FPN top-k,
                 ROI-align bilinear pooling (numpy fp32, exact formula match).
  device call 2: fc6 (K-sharded over 8 cores) + AllReduce + fc7 (N-sharded)
                 + AllGather + cls/bbox heads (N-sharded).
"""
import numpy as np

import concourse.bass as bass
import concourse.mybir as mybir
import concourse.tile as tile
from concourse import bacc
from concourse.bass_utils import run_bass_kernel_spmd

F32 = np.float32
DT = mybir.dt.float32
NCORES = 8

IMG_H, IMG_W = 800, 1280
STRIDES = (4, 8, 16, 32, 64)
SIZES = (32, 64, 128, 256, 512)
FEAT_HW = ((200, 320), (100, 160), (50, 80), (25, 40), (13, 20))
A = 3
PRE_NMS, POST_NMS, FPN_POST = 1000, 1000, 1000
NMS_TH = 0.7
POOL, SR = 7, 2
C = 256

# per-level sharding geometry: (H, W, Hc=rows/core, r=rows/group, G=groups/core)
GEOM = []
for (H, W) in FEAT_HW:
    Hc = -(-H // NCORES)          # ceil
    r = max(1, 512 // W)          # rows per PSUM group (N = r*W <= 512)
    r = min(r, Hc)
    G = -(-Hc // r)
    GEOM.append((H, W, Hc, r, G))
# -> p2:(200,320,25,1,25) p3:(100,160,13,3,5) p4:(50,80,7,6,2) p5:(25,40,4,4,1) p6:(13,20,2,2,1)

LEVELS = range(5)

_prog_cache = {}


# ---------------------------------------------------------------- call 1: RPN
def build_rpn_program():
    nc = bacc.Bacc(None, target_bir_lowering=False)
    xs, outs = [], []
    for l in LEVELS:
        H, W, Hc, r, G = GEOM[l]
        slab = G * r + 2
        xs.append(nc.declare_dram_parameter(f"x{l}", [2, 128, slab, W + 2], DT, isOutput=False))
    wconv = nc.declare_dram_parameter("wconv", [128, 9, 2, 256], DT, isOutput=False)
    whead = nc.declare_dram_parameter("whead", [128, 2, 15], DT, isOutput=False)
    cb = nc.declare_dram_parameter("cb", [128, 2], DT, isOutput=False)
    hb = nc.declare_dram_parameter("hb", [15, 1], DT, isOutput=False)
    for l in LEVELS:
        H, W, Hc, r, G = GEOM[l]
        outs.append(nc.declare_dram_parameter(f"o{l}", [G, 15, r * W], DT, isOutput=True))

    with tile.TileContext(nc) as tc:
        with (
            tc.tile_pool(name="const", bufs=1) as const_pool,
            tc.tile_pool(name="slabs", bufs=1) as slab_pool,
            tc.tile_pool(name="t", bufs=4) as t_pool,
            tc.tile_pool(name="ob", bufs=4) as out_pool,
            tc.tile_pool(name="ps", bufs=4, space="PSUM") as psum_pool,
            tc.tile_pool(name="ph", bufs=2, space="PSUM") as psum_head_pool,
        ):
            wc_sb = const_pool.tile([128, 9, 2, 256], DT)
            nc.sync.dma_start(wc_sb[:], wconv[:])
            wh_sb = const_pool.tile([128, 2, 15], DT)
            nc.sync.dma_start(wh_sb[:], whead[:])
            cb_sb = const_pool.tile([128, 2], DT)
            nc.sync.dma_start(cb_sb[:], cb[:])
            hb_sb = const_pool.tile([15, 1], DT)
            nc.sync.dma_start(hb_sb[:], hb[:])

            slab_sb = {}
            # issue loads small->large so small levels can start while p2 streams
            for l in [4, 3, 2, 1, 0]:
                H, W, Hc, r, G = GEOM[l]
                slab = G * r + 2
                s_sb = slab_pool.tile([128, 2, slab, W + 2], DT, tag=f"slab{l}")
                for kt in range(2):
                    nc.sync.dma_start(s_sb[:, kt], xs[l][kt])
                slab_sb[l] = s_sb

            for l in [4, 3, 2, 1, 0]:
                H, W, Hc, r, G = GEOM[l]
                N = r * W
                for g in range(G):
                    t_halves = []
                    for half in range(2):
                        ps = psum_pool.tile([128, N], DT, tag="ps")
                        for s in range(9):
                            di, dj = s // 3, s % 3
                            nc.tensor.matmul(
                                ps[:],
                                wc_sb[:, s, 0, half * 128:(half + 1) * 128],
                                slab_sb[l][:, 0, g * r + di:g * r + di + r, dj:dj + W],
                                start=(s == 0), stop=False,
                            )
                            nc.tensor.matmul(
                                ps[:],
                                wc_sb[:, s, 1, half * 128:(half + 1) * 128],
                                slab_sb[l][:, 1, g * r + di:g * r + di + r, dj:dj + W],
                                start=False, stop=(s == 8),
                            )
                        tt = t_pool.tile([128, N], DT, tag="t")
                        nc.scalar.activation(tt[:], ps[:],
                                             mybir.ActivationFunctionType.Relu,
                                             bias=cb_sb[:, half:half + 1])
                        t_halves.append(tt)
                    ph = psum_head_pool.tile([15, N], DT, tag="ph")
                    for kt in range(2):
                        nc.tensor.matmul(ph[:], wh_sb[:, kt, :], t_halves[kt][:],
                                         start=(kt == 0), stop=(kt == 1))
                    ob = out_pool.tile([15, N], DT, tag="ob")
                    nc.vector.tensor_add(ob[:], ph[:], hb_sb.to_broadcast([15, N]))
                    nc.sync.dma_start(outs[l][g], ob[:])
    nc.compile()
    return nc


def rpn_in_maps(feats, rpn_conv_w, rpn_cls_w, rpn_bbox_w, rpn_conv_b, rpn_cls_b, rpn_bbox_b):
    w_r = rpn_conv_w.reshape(256, 2, 128, 3, 3)
    wconv_arr = np.ascontiguousarray(
        np.transpose(w_r, (2, 3, 4, 1, 0)).reshape(128, 9, 2, 256), F32)
    hw = np.concatenate([rpn_cls_w[:, :, 0, 0], rpn_bbox_w[:, :, 0, 0]], 0)  # [15,256]
    whead_arr = np.ascontiguousarray(hw.reshape(15, 2, 128).transpose(2, 1, 0), F32)
    cb_arr = np.ascontiguousarray(rpn_conv_b.reshape(2, 128).T, F32)
    hb_arr = np.concatenate([rpn_cls_b, rpn_bbox_b])[:, None].astype(F32)

    slabs_per_core = [[] for _ in range(NCORES)]
    for l in LEVELS:
        H, W, Hc, r, G = GEOM[l]
        slab = G * r + 2
        Hbig = (NCORES - 1) * Hc + slab
        xp = np.zeros((2, 128, Hbig, W + 2), F32)
        xp[:, :, 1:H + 1, 1:W + 1] = feats[l][0].reshape(2, 128, H, W)
        for c in range(NCORES):
            slabs_per_core[c].append(
                np.ascontiguousarray(xp[:, :, c * Hc:c * Hc + slab]))
    in_maps = []
    for c in range(NCORES):
        m = {f"x{l}": slabs_per_core[c][l] for l in LEVELS}
        m.update(wconv=wconv_arr, whead=whead_arr, cb=cb_arr, hb=hb_arr)
        in_maps.append(m)
    return in_maps


def rpn_assemble(results):
    """per-core outputs -> full cls [A,H,W] and bbox [12,H,W] maps per level"""
    cls_maps, bbox_maps = [], []
    for l in LEVELS:
        H, W, Hc, r, G = GEOM[l]
        full = np.empty((15, H, W), F32)
        for c in range(NCORES):
            o = results[c][f"o{l}"].reshape(G, 15, r, W).transpose(1, 0, 2, 3).reshape(15, G * r, W)
            lo = c * Hc
            hi = min(H, lo + Hc)
            if lo >= H:
                continue
            full[:, lo:hi] = o[:, :hi - lo]
        cls_maps.append(full[:3])
        bbox_maps.append(full[3:])
    return cls_maps, bbox_maps


# ------------------------------------------------------------- host pipeline
def make_anchors(size, stride, H, W):
    ratios = np.array([0.5, 1.0, 2.0], F32)
    ws = np.sqrt(F32(size * size) / ratios)
    hs = ws * ratios
    base = np.stack([-ws / 2, -hs / 2, ws / 2, hs / 2], axis=1).astype(F32)
    X, Y = np.meshgrid(np.arange(W, dtype=F32) * F32(stride),
                       np.arange(H, dtype=F32) * F32(stride))
    shifts = np.stack([X, Y, X, Y], axis=-1).reshape(-1, 4).astype(F32)
    return (shifts[:, None, :] + base[None, :, :]).reshape(-1, 4)


def decode(rel, boxes):
    w = boxes[:, 2] - boxes[:, 0] + F32(1.0)
    h = boxes[:, 3] - boxes[:, 1] + F32(1.0)
    cx = boxes[:, 0] + F32(0.5) * w
    cy = boxes[:, 1] + F32(0.5) * h
    dx, dy, dw, dh = rel[:, 0], rel[:, 1], rel[:, 2], rel[:, 3]
    bbox_clip = np.log(F32(1000.0 / 16)).astype(F32)
    dw = np.minimum(dw, bbox_clip)
    dh = np.minimum(dh, bbox_clip)
    pcx = dx * w + cx
    pcy = dy * h + cy
    pw = np.exp(dw) * w
    ph = np.exp(dh) * h
    return np.stack([pcx - F32(0.5) * pw, pcy - F32(0.5) * ph,
                     pcx + F32(0.5) * pw - 1, pcy + F32(0.5) * ph - 1], axis=1)


def clip_boxes(b):
    return np.stack([np.clip(b[:, 0], 0, IMG_W - 1), np.clip(b[:, 1], 0, IMG_H - 1),
                     np.clip(b[:, 2], 0, IMG_W - 1), np.clip(b[:, 3], 0, IMG_H - 1)], axis=1)


def box_iou_one(box, boxes):
    lt = np.maximum(box[:2], boxes[:, :2])
    rb = np.minimum(box[2:], boxes[:, 2:])
    wh = np.clip(rb - lt + F32(1.0), 0, None)
    inter = wh[:, 0] * wh[:, 1]
    a1 = (box[2] - box[0] + F32(1.0)) * (box[3] - box[1] + F32(1.0))
    a2 = (boxes[:, 2] - boxes[:, 0] + F32(1.0)) * (boxes[:, 3] - boxes[:, 1] + F32(1.0))
    return inter / (a1 + a2 - inter)


def nms(boxes, scores, iou_th, max_out):
    order = np.argsort(-scores, kind="stable")
    b = boxes[order]
    n = b.shape[0]
    sup = np.zeros((n,), bool)
    keep = np.full((max_out,), -1, np.int32)
    cnt = 0
    for _ in range(max_out):
        nsup = ~sup
        if not nsup.any():
            break
        idx = int(np.argmax(nsup))
        ious = box_iou_one(b[idx], b)
        sup |= ious > F32(iou_th)
        keep[cnt] = order[idx]
        cnt += 1
    return keep


def sigmoid(x):
    out = np.empty_like(x)
    pos = x >= 0
    out[pos] = F32(1.0) / (F32(1.0) + np.exp(-x[pos]))
    ex = np.exp(x[~pos])
    out[~pos] = ex / (F32(1.0) + ex)
    return out


def level_proposals(cls_map, bbox_map, anchors):
    H, W = cls_map.shape[1], cls_map.shape[2]
    logits = np.transpose(cls_map, (1, 2, 0)).reshape(-1)
    deltas = np.transpose(bbox_map.reshape(A, 4, H, W), (2, 3, 0, 1)).reshape(-1, 4)
    k = min(PRE_NMS, logits.shape[0])
    order = np.argsort(-logits, kind="stable")[:k]
    top_logits = logits[order]
    boxes = clip_boxes(decode(deltas[order], anchors[order]))
    keep = nms(boxes, top_logits, NMS_TH, POST_NMS)
    valid = keep >= 0
    ki = np.maximum(keep, 0)
    out_boxes = np.where(valid[:, None], boxes[ki], F32(0.0)).astype(F32)
    out_scores = np.where(valid, sigmoid(top_logits[ki]), F32(-1.0)).astype(F32)
    return out_boxes, out_scores


def rpn_proposals(cls_maps, bbox_maps):
    all_boxes, all_scores = [], []
    for l, (stride, size, (H, W)) in enumerate(zip(STRIDES, SIZES, FEAT_HW)):
        anchors = make_anchors(size, stride, H, W)
        b, s = level_proposals(cls_maps[l], bbox_maps[l], anchors)
        all_boxes.append(b)
        all_scores.append(s)
    boxes = np.concatenate(all_boxes, 0)
    scores = np.concatenate(all_scores, 0)
    top = np.argsort(-scores, kind="stable")[:FPN_POST]
    return boxes[top]


def bilinear(feat, ys, xs):
    Cc, H, W = feat.shape
    N, S = ys.shape
    y = np.broadcast_to(ys[:, :, None], (N, S, S))
    x = np.broadcast_to(xs[:, None, :], (N, S, S))
    mask = (y > -1.0) & (y < H) & (x > -1.0) & (x < W)
    y = np.clip(y, F32(0.0), F32(H - 1))
    x = np.clip(x, F32(0.0), F32(W - 1))
    y0f = np.floor(y)
    x0f = np.floor(x)
    y0 = y0f.astype(np.int32)
    x0 = x0f.astype(np.int32)
    y1 = np.minimum(y0 + 1, H - 1)
    x1 = np.minimum(x0 + 1, W - 1)
    ly = (y - y0f).astype(F32)
    lx = (x - x0f).astype(F32)
    hy = F32(1) - ly
    hx = F32(1) - lx
    fl = feat.reshape(Cc, H * W)
    i00 = (y0 * W + x0).ravel()
    i01 = (y0 * W + x1).ravel()
    i10 = (y1 * W + x0).ravel()
    i11 = (y1 * W + x1).ravel()
    v00 = fl[:, i00].reshape(Cc, N, S, S)
    v01 = fl[:, i01].reshape(Cc, N, S, S)
    v10 = fl[:, i10].reshape(Cc, N, S, S)
    v11 = fl[:, i11].reshape(Cc, N, S, S)
    val = (hy * hx)[None] * v00 + (hy * lx)[None] * v01 + \
          (ly * hx)[None] * v10 + (ly * lx)[None] * v11
    val = np.where(mask[None], val, F32(0.0))
    return np.transpose(val, (1, 0, 2, 3))


def roi_align(feat, rois, scale):
    N = rois.shape[0]
    x1 = rois[:, 0] * F32(scale)
    y1 = rois[:, 1] * F32(scale)
    rw = np.maximum(rois[:, 2] * F32(scale) - x1, F32(1.0))
    rh = np.maximum(rois[:, 3] * F32(scale) - y1, F32(1.0))
    g = ((np.arange(POOL * SR, dtype=F32) + F32(0.5)) / F32(SR)).astype(F32)
    xs = x1[:, None] + g[None, :] * (rw / F32(POOL))[:, None]
    ys = y1[:, None] + g[None, :] * (rh / F32(POOL))[:, None]
    v = bilinear(feat, ys, xs)
    return v.reshape(N, v.shape[1], POOL, SR, POOL, SR).mean(axis=(3, 5), dtype=F32)


def pooled_features(feats, proposals):
    area = np.maximum((proposals[:, 2] - proposals[:, 0]) *
                      (proposals[:, 3] - proposals[:, 1]), F32(1e-6))
    lvl = np.floor(F32(4) + np.log2(np.sqrt(area) / F32(224.0) + F32(1e-6)))
    lvl = np.clip(lvl, 2, 5).astype(np.int32) - 2
    pooled = np.zeros((proposals.shape[0], C, POOL, POOL), F32)
    for l in range(4):
        sel = np.nonzero(lvl == l)[0]
        if sel.size == 0:
            continue
        pooled[sel] = roi_align(feats[l], proposals[sel], 1.0 / STRIDES[l])
    return pooled.reshape(proposals.shape[0], -1)


# ---------------------------------------------------------------- call 2: FC
KT6 = 13          # k-tiles of 128 per core for fc6 (13*128*8 = 13312 >= 12544)


def build_fc_program():
    nc = bacc.Bacc(None, target_bir_lowering=False)
    xk = nc.declare_dram_parameter("xk", [KT6, 128, 1000], DT, isOutput=False)
    w6k = nc.declare_dram_parameter("w6k", [KT6, 128, 8, 128], DT, isOutput=False)
    b6 = nc.declare_dram_parameter("b6", [8, 128, 1], DT, isOutput=False)
    w7k = nc.declare_dram_parameter("w7k", [8, 128, 128], DT, isOutput=False)
    b7 = nc.declare_dram_parameter("b7", [128, 1], DT, isOutput=False)
    whk = nc.declare_dram_parameter("whk", [8, 128, 51], DT, isOutput=False)
    bh = nc.declare_dram_parameter("bh", [51, 1], DT, isOutput=False)
    oh = nc.declare_dram_parameter("oh", [51, 1000], DT, isOutput=True)

    groups = [list(range(NCORES))]
    with tile.TileContext(nc) as tc:
        with (
            tc.tile_pool(name="big", bufs=1) as big_pool,
            tc.tile_pool(name="work", bufs=2) as work_pool,
            tc.tile_pool(name="ps", bufs=2, space="PSUM") as psum_pool,
            tc.tile_pool(name="dram", bufs=1, space="DRAM") as dram_pool,
        ):
            xk_sb = big_pool.tile([128, KT6, 1000], DT)
            for kt in range(KT6):
                nc.sync.dma_start(xk_sb[:, kt], xk[kt])
            w6_sb = big_pool.tile([128, KT6, 8, 128], DT)
            for kt in range(KT6):
                nc.sync.dma_start(w6_sb[:, kt], w6k[kt])
            b6_sb = big_pool.tile([128, 8], DT)
            for m8 in range(8):
                nc.sync.dma_start(b6_sb[:, m8:m8 + 1], b6[m8])
            w7_sb = big_pool.tile([128, 8, 128], DT)
            for kt in range(8):
                nc.sync.dma_start(w7_sb[:, kt], w7k[kt])
            b7_sb = big_pool.tile([128, 1], DT)
            nc.sync.dma_start(b7_sb[:], b7[:])
            wh_sb = big_pool.tile([128, 8, 51], DT)
            for kt in range(8):
                nc.sync.dma_start(wh_sb[:, kt], whk[kt])
            bh_sb = big_pool.tile([51, 1], DT)
            nc.sync.dma_start(bh_sb[:], bh[:])

            z6p = dram_pool.tile([8, 128, 1000], DT)
            z6r = dram_pool.tile([8, 128, 1000], DT)
            x8p = dram_pool.tile([128, 1000], DT)
            x8g = dram_pool.tile([8, 128, 1000], DT)

            # fc6 partials (K-sharded): z6p[m8] = W6_slice.T @ pooled_slice.T
            for m8 in range(8):
                for nn in range(2):
                    ps = psum_pool.tile([128, 500], DT, tag="ps6")
                    for kt in range(KT6):
                        nc.tensor.matmul(ps[:], w6_sb[:, kt, m8, :],
                                         xk_sb[:, kt, nn * 500:(nn + 1) * 500],
                                         start=(kt == 0), stop=(kt == KT6 - 1))
                    zt = work_pool.tile([128, 500], DT, tag="z6")
                    nc.vector.tensor_copy(zt[:], ps[:])
                    nc.sync.dma_start(z6p[m8, :, nn * 500:(nn + 1) * 500], zt[:])

            nc.gpsimd.collective_compute(
                "AllReduce", mybir.AluOpType.add, replica_groups=groups,
                ins=[z6p.opt()], outs=[z6r.opt()])

            # x7 = relu(z6 + b6), all 8 m-tiles kept in SBUF
            x7_sb = big_pool.tile([128, 8, 1000], DT, tag="x78")
            for m8 in range(8):
                zin = work_pool.tile([128, 1000], DT, tag="zin")
                nc.sync.dma_start(zin[:], z6r[m8])
                nc.scalar.activation(x7_sb[:, m8], zin[:],
                                     mybir.ActivationFunctionType.Relu,
                                     bias=b6_sb[:, m8:m8 + 1])

            # fc7 N-shard: this core's 128 outputs over all 1000 rois
            x8_sb = big_pool.tile([128, 1000], DT)
            for nn in range(2):
                ps = psum_pool.tile([128, 500], DT, tag="ps7")
                for kt in range(8):
                    nc.tensor.matmul(ps[:], w7_sb[:, kt, :],
                                     x7_sb[:, kt, nn * 500:(nn + 1) * 500],
                                     start=(kt == 0), stop=(kt == 7))
                nc.scalar.activation(x8_sb[:, nn * 500:(nn + 1) * 500], ps[:],
                                     mybir.ActivationFunctionType.Relu,
                                     bias=b7_sb[:])
            nc.sync.dma_start(x8p[:], x8_sb[:])

            nc.gpsimd.collective_compute(
                "AllGather", mybir.AluOpType.bypass, replica_groups=groups,
                ins=[x8p.opt()], outs=[x8g.opt()])

            # heads N-shard: 51 output cols per core
            x8f_sb = big_pool.tile([128, 8, 1000], DT, tag="x78")
            for kt in range(8):
                nc.sync.dma_start(x8f_sb[:, kt], x8g[kt])
            oh_sb = work_pool.tile([51, 1000], DT, tag="oh")
            for nn in range(2):
                ps = psum_pool.tile([51, 500], DT, tag="psh")
                for kt in range(8):
                    nc.tensor.matmul(ps[:], wh_sb[:, kt, :],
                                     x8f_sb[:, kt, nn * 500:(nn + 1) * 500],
                                     start=(kt == 0), stop=(kt == 7))
                nc.vector.tensor_add(oh_sb[:, nn * 500:(nn + 1) * 500], ps[:],
                                     bh_sb.to_broadcast([51, 500]))
            nc.sync.dma_start(oh[:], oh_sb[:])
    nc.compile()
    return nc


def fc_in_maps(pooled, fc6_w, fc6_b, fc7_w, fc7_b, cls_w, cls_b, bbox_w, bbox_b):
    K6 = KT6 * 128 * NCORES
    pooledT = np.zeros((K6, 1000), F32)
    pooledT[:12544] = pooled.T
    w6T = np.zeros((K6, 1024), F32)
    w6T[:12544] = fc6_w.T
    wcb = np.zeros((408, 1024), F32)
    wcb[:81] = cls_w
    wcb[81:405] = bbox_w
    bcb = np.zeros((408,), F32)
    bcb[:81] = cls_b
    bcb[81:405] = bbox_b
    b6_arr = fc6_b.reshape(8, 128, 1).astype(F32)
    in_maps = []
    for c in range(NCORES):
        ks = slice(c * KT6 * 128, (c + 1) * KT6 * 128)
        m = dict(
            xk=np.ascontiguousarray(pooledT[ks].reshape(KT6, 128, 1000)),
            w6k=np.ascontiguousarray(w6T[ks].reshape(KT6, 128, 8, 128)),
            b6=b6_arr,
            w7k=np.ascontiguousarray(fc7_w[c * 128:(c + 1) * 128, :].T.reshape(8, 128, 128)),
            b7=fc7_b[c * 128:(c + 1) * 128].reshape(128, 1).astype(F32),
            whk=np.ascontiguousarray(wcb[c * 51:(c + 1) * 51].T.reshape(8, 128, 51)),
            bh=bcb[c * 51:(c + 1) * 51].reshape(51, 1),
        )
        in_maps.append(m)
    return in_maps


# ------------------------------------------------------------------- kernel
LAST_TIMES = {}


def kernel(p2, p3, p4, p5, p6, rpn_conv_w, rpn_conv_b, rpn_cls_w, rpn_cls_b,
           rpn_bbox_w, rpn_bbox_b, fc6_w, fc6_b, fc7_w, fc7_b,
           cls_w, cls_b, bbox_w, bbox_b):
    import time
    feats = [np.asarray(x, F32) for x in (p2, p3, p4, p5, p6)]

    if "rpn" not in _prog_cache:
        _prog_cache["rpn"] = build_rpn_program()
    if "fc" not in _prog_cache:
        _prog_cache["fc"] = build_fc_program()

    t0 = time.time()
    maps1 = rpn_in_maps(feats, np.asarray(rpn_conv_w, F32), np.asarray(rpn_cls_w, F32),
                        np.asarray(rpn_bbox_w, F32), np.asarray(rpn_conv_b, F32),
                        np.asarray(rpn_cls_b, F32), np.asarray(rpn_bbox_b, F32))
    t1 = time.time()
    res1 = run_bass_kernel_spmd(_prog_cache["rpn"], maps1, core_ids=list(range(NCORES)))
    t2 = time.time()
    cls_maps, bbox_maps = rpn_assemble(res1.results)
    props = rpn_proposals(cls_maps, bbox_maps)
    t3 = time.time()
    pooled = pooled_features([f[0] for f in feats[:4]], props)
    t4 = time.time()
    maps2 = fc_in_maps(pooled, np.asarray(fc6_w, F32), np.asarray(fc6_b, F32),
                       np.asarray(fc7_w, F32), np.asarray(fc7_b, F32),
                       np.asarray(cls_w, F32), np.asarray(cls_b, F32),
                       np.asarray(bbox_w, F32), np.asarray(bbox_b, F32))
    t5 = time.time()
    res2 = run_bass_kernel_spmd(_prog_cache["fc"], maps2, core_ids=list(range(NCORES)))
    t6 = time.time()
    ohs = np.stack([res2.results[c]["oh"] for c in range(NCORES)])  # [8,51,1000]
    outT = ohs.reshape(408, 1000)
    class_logits = np.ascontiguousarray(outT[:81].T)
    box_regression = np.ascontiguousarray(outT[81:405].T)
    LAST_TIMES.update(prep1=t1 - t0, call1=t2 - t1, host_props=t3 - t2,
                      roi_align=t4 - t3, prep2=t5 - t4, call2=t6 - t5)
    return class_logits, box_regression, props


# revision 4
# speedup vs baseline: 1.0525x; 1.0525x over previous
"""Faster R-CNN detection head (RPN + NMS + ROI-align + box head) on 8 trn2 cores.

Split:
  device call 1: RPN 3x3 conv + ReLU + 1x1 cls/bbox heads over 5 FPN levels
                 (rows sharded across 8 cores; conv = 9 shifted matmuls
                 accumulated in PSUM).
  host:          per-level top-k, box decode, greedy NMS, FPN top-k,
                 ROI-align bilinear pooling (numpy fp32, exact formula match).
  device call 2: fc6 (K-sharded over 8 cores) + AllReduce + fc7 (N-sharded)
                 + AllGather + cls/bbox heads (N-sharded).
"""
import numpy as np

import concourse.bass as bass
import concourse.mybir as mybir
import concourse.tile as tile
from concourse import bacc
from concourse.bass_utils import run_bass_kernel_spmd

F32 = np.float32
DT = mybir.dt.float32
NCORES = 8

IMG_H, IMG_W = 800, 1280
STRIDES = (4, 8, 16, 32, 64)
SIZES = (32, 64, 128, 256, 512)
FEAT_HW = ((200, 320), (100, 160), (50, 80), (25, 40), (13, 20))
A = 3
PRE_NMS, POST_NMS, FPN_POST = 1000, 1000, 1000
NMS_TH = 0.7
POOL, SR = 7, 2
C = 256

# per-level sharding geometry: (H, W, Hc=rows/core, r=rows/group, G=groups/core)
GEOM = []
for (H, W) in FEAT_HW:
    Hc = -(-H // NCORES)          # ceil
    r = max(1, 512 // W)          # rows per PSUM group (N = r*W <= 512)
    r = min(r, Hc)
    G = -(-Hc // r)
    GEOM.append((H, W, Hc, r, G))
# -> p2:(200,320,25,1,25) p3:(100,160,13,3,5) p4:(50,80,7,6,2) p5:(25,40,4,4,1) p6:(13,20,2,2,1)

LEVELS = range(5)

_prog_cache = {}


# ---------------------------------------------------------------- call 1: RPN
def build_rpn_program():
    nc = bacc.Bacc(None, target_bir_lowering=False)
    xs, outs = [], []
    for l in LEVELS:
        H, W, Hc, r, G = GEOM[l]
        slab = G * r + 2
        xs.append(nc.declare_dram_parameter(f"x{l}", [2, 128, slab, W + 2], DT, isOutput=False))
    wconv = nc.declare_dram_parameter("wconv", [128, 9, 2, 256], DT, isOutput=False)
    whead = nc.declare_dram_parameter("whead", [128, 2, 15], DT, isOutput=False)
    cb = nc.declare_dram_parameter("cb", [128, 2], DT, isOutput=False)
    hb = nc.declare_dram_parameter("hb", [15, 1], DT, isOutput=False)
    for l in LEVELS:
        H, W, Hc, r, G = GEOM[l]
        outs.append(nc.declare_dram_parameter(f"o{l}", [G, 15, r * W], DT, isOutput=True))

    with tile.TileContext(nc) as tc:
        with (
            tc.tile_pool(name="const", bufs=1) as const_pool,
            tc.tile_pool(name="slabs", bufs=1) as slab_pool,
            tc.tile_pool(name="t", bufs=4) as t_pool,
            tc.tile_pool(name="ob", bufs=4) as out_pool,
            tc.tile_pool(name="ps", bufs=4, space="PSUM") as psum_pool,
            tc.tile_pool(name="ph", bufs=2, space="PSUM") as psum_head_pool,
        ):
            wc_sb = const_pool.tile([128, 9, 2, 256], DT)
            nc.sync.dma_start(wc_sb[:], wconv[:])
            wh_sb = const_pool.tile([128, 2, 15], DT)
            nc.sync.dma_start(wh_sb[:], whead[:])
            cb_sb = const_pool.tile([128, 2], DT)
            nc.sync.dma_start(cb_sb[:], cb[:])
            hb_sb = const_pool.tile([15, 1], DT)
            nc.sync.dma_start(hb_sb[:], hb[:])

            slab_sb = {}
            # issue loads small->large so small levels can start while p2 streams
            for l in [4, 3, 2, 1, 0]:
                H, W, Hc, r, G = GEOM[l]
                slab = G * r + 2
                s_sb = slab_pool.tile([128, 2, slab, W + 2], DT, tag=f"slab{l}")
                for kt in range(2):
                    nc.sync.dma_start(s_sb[:, kt], xs[l][kt])
                slab_sb[l] = s_sb

            for l in [4, 3, 2, 1, 0]:
                H, W, Hc, r, G = GEOM[l]
                N = r * W
                for g in range(G):
                    t_halves = []
                    for half in range(2):
                        ps = psum_pool.tile([128, N], DT, tag="ps")
                        for s in range(9):
                            di, dj = s // 3, s % 3
                            nc.tensor.matmul(
                                ps[:],
                                wc_sb[:, s, 0, half * 128:(half + 1) * 128],
                                slab_sb[l][:, 0, g * r + di:g * r + di + r, dj:dj + W],
                                start=(s == 0), stop=False,
                            )
                            nc.tensor.matmul(
                                ps[:],
                                wc_sb[:, s, 1, half * 128:(half + 1) * 128],
                                slab_sb[l][:, 1, g * r + di:g * r + di + r, dj:dj + W],
                                start=False, stop=(s == 8),
                            )
                        tt = t_pool.tile([128, N], DT, tag="t")
                        nc.scalar.activation(tt[:], ps[:],
                                             mybir.ActivationFunctionType.Relu,
                                             bias=cb_sb[:, half:half + 1])
                        t_halves.append(tt)
                    ph = psum_head_pool.tile([15, N], DT, tag="ph")
                    for kt in range(2):
                        nc.tensor.matmul(ph[:], wh_sb[:, kt, :], t_halves[kt][:],
                                         start=(kt == 0), stop=(kt == 1))
                    ob = out_pool.tile([15, N], DT, tag="ob")
                    nc.vector.tensor_add(ob[:], ph[:], hb_sb.to_broadcast([15, N]))
                    nc.sync.dma_start(outs[l][g], ob[:])
    nc.compile()
    return nc


def rpn_in_maps(feats, rpn_conv_w, rpn_cls_w, rpn_bbox_w, rpn_conv_b, rpn_cls_b, rpn_bbox_b):
    w_r = rpn_conv_w.reshape(256, 2, 128, 3, 3)
    wconv_arr = np.ascontiguousarray(
        np.transpose(w_r, (2, 3, 4, 1, 0)).reshape(128, 9, 2, 256), F32)
    hw = np.concatenate([rpn_cls_w[:, :, 0, 0], rpn_bbox_w[:, :, 0, 0]], 0)  # [15,256]
    whead_arr = np.ascontiguousarray(hw.reshape(15, 2, 128).transpose(2, 1, 0), F32)
    cb_arr = np.ascontiguousarray(rpn_conv_b.reshape(2, 128).T, F32)
    hb_arr = np.concatenate([rpn_cls_b, rpn_bbox_b])[:, None].astype(F32)

    slabs_per_core = [[] for _ in range(NCORES)]
    for l in LEVELS:
        H, W, Hc, r, G = GEOM[l]
        slab = G * r + 2
        Hbig = (NCORES - 1) * Hc + slab
        xp = np.zeros((2, 128, Hbig, W + 2), F32)
        xp[:, :, 1:H + 1, 1:W + 1] = feats[l][0].reshape(2, 128, H, W)
        for c in range(NCORES):
            slabs_per_core[c].append(
                np.ascontiguousarray(xp[:, :, c * Hc:c * Hc + slab]))
    in_maps = []
    for c in range(NCORES):
        m = {f"x{l}": slabs_per_core[c][l] for l in LEVELS}
        m.update(wconv=wconv_arr, whead=whead_arr, cb=cb_arr, hb=hb_arr)
        in_maps.append(m)
    return in_maps


def rpn_assemble(results):
    """per-core outputs -> full cls [A,H,W] and bbox [12,H,W] maps per level"""
    cls_maps, bbox_maps = [], []
    for l in LEVELS:
        H, W, Hc, r, G = GEOM[l]
        full = np.empty((15, H, W), F32)
        for c in range(NCORES):
            o = results[c][f"o{l}"].reshape(G, 15, r, W).transpose(1, 0, 2, 3).reshape(15, G * r, W)
            lo = c * Hc
            hi = min(H, lo + Hc)
            if lo >= H:
                continue
            full[:, lo:hi] = o[:, :hi - lo]
        cls_maps.append(full[:3])
        bbox_maps.append(full[3:])
    return cls_maps, bbox_maps


# ------------------------------------------------------------- host pipeline
def make_anchors(size, stride, H, W):
    ratios = np.array([0.5, 1.0, 2.0], F32)
    ws = np.sqrt(F32(size * size) / ratios)
    hs = ws * ratios
    base = np.stack([-ws / 2, -hs / 2, ws / 2, hs / 2], axis=1).astype(F32)
    X, Y = np.meshgrid(np.arange(W, dtype=F32) * F32(stride),
                       np.arange(H, dtype=F32) * F32(stride))
    shifts = np.stack([X, Y, X, Y], axis=-1).reshape(-1, 4).astype(F32)
    return (shifts[:, None, :] + base[None, :, :]).reshape(-1, 4)


def decode(rel, boxes):
    w = boxes[:, 2] - boxes[:, 0] + F32(1.0)
    h = boxes[:, 3] - boxes[:, 1] + F32(1.0)
    cx = boxes[:, 0] + F32(0.5) * w
    cy = boxes[:, 1] + F32(0.5) * h
    dx, dy, dw, dh = rel[:, 0], rel[:, 1], rel[:, 2], rel[:, 3]
    bbox_clip = np.log(F32(1000.0 / 16)).astype(F32)
    dw = np.minimum(dw, bbox_clip)
    dh = np.minimum(dh, bbox_clip)
    pcx = dx * w + cx
    pcy = dy * h + cy
    pw = np.exp(dw) * w
    ph = np.exp(dh) * h
    return np.stack([pcx - F32(0.5) * pw, pcy - F32(0.5) * ph,
                     pcx + F32(0.5) * pw - 1, pcy + F32(0.5) * ph - 1], axis=1)


def clip_boxes(b):
    return np.stack([np.clip(b[:, 0], 0, IMG_W - 1), np.clip(b[:, 1], 0, IMG_H - 1),
                     np.clip(b[:, 2], 0, IMG_W - 1), np.clip(b[:, 3], 0, IMG_H - 1)], axis=1)


def box_iou_one(box, boxes):
    lt = np.maximum(box[:2], boxes[:, :2])
    rb = np.minimum(box[2:], boxes[:, 2:])
    wh = np.clip(rb - lt + F32(1.0), 0, None)
    inter = wh[:, 0] * wh[:, 1]
    a1 = (box[2] - box[0] + F32(1.0)) * (box[3] - box[1] + F32(1.0))
    a2 = (boxes[:, 2] - boxes[:, 0] + F32(1.0)) * (boxes[:, 3] - boxes[:, 1] + F32(1.0))
    return inter / (a1 + a2 - inter)


def nms(boxes, scores, iou_th, max_out):
    order = np.argsort(-scores, kind="stable")
    b = boxes[order]
    n = b.shape[0]
    sup = np.zeros((n,), bool)
    keep = np.full((max_out,), -1, np.int32)
    cnt = 0
    for _ in range(max_out):
        nsup = ~sup
        if not nsup.any():
            break
        idx = int(np.argmax(nsup))
        ious = box_iou_one(b[idx], b)
        sup |= ious > F32(iou_th)
        keep[cnt] = order[idx]
        cnt += 1
    return keep


def sigmoid(x):
    out = np.empty_like(x)
    pos = x >= 0
    out[pos] = F32(1.0) / (F32(1.0) + np.exp(-x[pos]))
    ex = np.exp(x[~pos])
    out[~pos] = ex / (F32(1.0) + ex)
    return out


def level_proposals(cls_map, bbox_map, anchors):
    H, W = cls_map.shape[1], cls_map.shape[2]
    logits = np.transpose(cls_map, (1, 2, 0)).reshape(-1)
    deltas = np.transpose(bbox_map.reshape(A, 4, H, W), (2, 3, 0, 1)).reshape(-1, 4)
    k = min(PRE_NMS, logits.shape[0])
    order = np.argsort(-logits, kind="stable")[:k]
    top_logits = logits[order]
    boxes = clip_boxes(decode(deltas[order], anchors[order]))
    keep = nms(boxes, top_logits, NMS_TH, POST_NMS)
    valid = keep >= 0
    ki = np.maximum(keep, 0)
    out_boxes = np.where(valid[:, None], boxes[ki], F32(0.0)).astype(F32)
    out_scores = np.where(valid, sigmoid(top_logits[ki]), F32(-1.0)).astype(F32)
    return out_boxes, out_scores


def rpn_proposals(cls_maps, bbox_maps):
    all_boxes, all_scores = [], []
    for l, (stride, size, (H, W)) in enumerate(zip(STRIDES, SIZES, FEAT_HW)):
        anchors = make_anchors(size, stride, H, W)
        b, s = level_proposals(cls_maps[l], bbox_maps[l], anchors)
        all_boxes.append(b)
        all_scores.append(s)
    boxes = np.concatenate(all_boxes, 0)
    scores = np.concatenate(all_scores, 0)
    top = np.argsort(-scores, kind="stable")[:FPN_POST]
    return boxes[top]


def roi_align(featT, H, W, rois, scale):
    # featT: [H*W, C] row-major (channel-last) for contiguous row gathers
    N = rois.shape[0]
    Cc = featT.shape[1]
    S = POOL * SR
    x1 = rois[:, 0] * F32(scale)
    y1 = rois[:, 1] * F32(scale)
    rw = np.maximum(rois[:, 2] * F32(scale) - x1, F32(1.0))
    rh = np.maximum(rois[:, 3] * F32(scale) - y1, F32(1.0))
    g = ((np.arange(S, dtype=F32) + F32(0.5)) / F32(SR)).astype(F32)
    xs = x1[:, None] + g[None, :] * (rw / F32(POOL))[:, None]
    ys = y1[:, None] + g[None, :] * (rh / F32(POOL))[:, None]
    y = np.broadcast_to(ys[:, :, None], (N, S, S))
    x = np.broadcast_to(xs[:, None, :], (N, S, S))
    mask = (y > -1.0) & (y < H) & (x > -1.0) & (x < W)
    y = np.clip(y, F32(0.0), F32(H - 1))
    x = np.clip(x, F32(0.0), F32(W - 1))
    y0f = np.floor(y)
    x0f = np.floor(x)
    y0 = y0f.astype(np.int32)
    x0 = x0f.astype(np.int32)
    y1i = np.minimum(y0 + 1, H - 1)
    x1i = np.minimum(x0 + 1, W - 1)
    ly = (y - y0f).astype(F32)
    lx = (x - x0f).astype(F32)
    hy = F32(1) - ly
    hx = F32(1) - lx
    m = mask.astype(F32)
    w00 = (hy * hx * m).reshape(-1, 1)
    w01 = (hy * lx * m).reshape(-1, 1)
    w10 = (ly * hx * m).reshape(-1, 1)
    w11 = (ly * lx * m).reshape(-1, 1)
    i00 = (y0 * W + x0).ravel()
    i01 = (y0 * W + x1i).ravel()
    i10 = (y1i * W + x0).ravel()
    i11 = (y1i * W + x1i).ravel()
    val = w00 * featT[i00]
    val += w01 * featT[i01]
    val += w10 * featT[i10]
    val += w11 * featT[i11]                     # [N*S*S, C]
    val = val.reshape(N, POOL, SR, POOL, SR, Cc).mean(axis=(2, 4), dtype=F32)
    return np.transpose(val, (0, 3, 1, 2))      # [N, C, 7, 7]


def pooled_features(feats, proposals):
    area = np.maximum((proposals[:, 2] - proposals[:, 0]) *
                      (proposals[:, 3] - proposals[:, 1]), F32(1e-6))
    lvl = np.floor(F32(4) + np.log2(np.sqrt(area) / F32(224.0) + F32(1e-6)))
    lvl = np.clip(lvl, 2, 5).astype(np.int32) - 2
    pooled = np.zeros((proposals.shape[0], C, POOL, POOL), F32)
    for l in range(4):
        sel = np.nonzero(lvl == l)[0]
        if sel.size == 0:
            continue
        H, W = FEAT_HW[l]
        featT = np.ascontiguousarray(feats[l].reshape(C, H * W).T)
        pooled[sel] = roi_align(featT, H, W, proposals[sel], 1.0 / STRIDES[l])
    return pooled.reshape(proposals.shape[0], -1)


# ---------------------------------------------------------------- call 2: FC
KT6 = 13          # k-tiles of 128 per core for fc6 (13*128*8 = 13312 >= 12544)


def build_fc_program():
    nc = bacc.Bacc(None, target_bir_lowering=False)
    xk = nc.declare_dram_parameter("xk", [KT6, 128, 1000], DT, isOutput=False)
    w6k = nc.declare_dram_parameter("w6k", [KT6, 128, 8, 128], DT, isOutput=False)
    b6 = nc.declare_dram_parameter("b6", [8, 128, 1], DT, isOutput=False)
    w7k = nc.declare_dram_parameter("w7k", [8, 128, 128], DT, isOutput=False)
    b7 = nc.declare_dram_parameter("b7", [128, 1], DT, isOutput=False)
    whk = nc.declare_dram_parameter("whk", [8, 128, 51], DT, isOutput=False)
    bh = nc.declare_dram_parameter("bh", [51, 1], DT, isOutput=False)
    oh = nc.declare_dram_parameter("oh", [51, 1000], DT, isOutput=True)

    groups = [list(range(NCORES))]
    with tile.TileContext(nc) as tc:
        with (
            tc.tile_pool(name="big", bufs=1) as big_pool,
            tc.tile_pool(name="work", bufs=2) as work_pool,
            tc.tile_pool(name="ps", bufs=2, space="PSUM") as psum_pool,
            tc.tile_pool(name="dram", bufs=1, space="DRAM") as dram_pool,
        ):
            xk_sb = big_pool.tile([128, KT6, 1000], DT)
            for kt in range(KT6):
                nc.sync.dma_start(xk_sb[:, kt], xk[kt])
            w6_sb = big_pool.tile([128, KT6, 8, 128], DT)
            for kt in range(KT6):
                nc.sync.dma_start(w6_sb[:, kt], w6k[kt])
            b6_sb = big_pool.tile([128, 8], DT)
            for m8 in range(8):
                nc.sync.dma_start(b6_sb[:, m8:m8 + 1], b6[m8])
            w7_sb = big_pool.tile([128, 8, 128], DT)
            for kt in range(8):
                nc.sync.dma_start(w7_sb[:, kt], w7k[kt])
            b7_sb = big_pool.tile([128, 1], DT)
            nc.sync.dma_start(b7_sb[:], b7[:])
            wh_sb = big_pool.tile([128, 8, 51], DT)
            for kt in range(8):
                nc.sync.dma_start(wh_sb[:, kt], whk[kt])
            bh_sb = big_pool.tile([51, 1], DT)
            nc.sync.dma_start(bh_sb[:], bh[:])

            z6p = dram_pool.tile([8, 128, 1000], DT)
            z6r = dram_pool.tile([8, 128, 1000], DT)
            x8p = dram_pool.tile([128, 1000], DT)
            x8g = dram_pool.tile([8, 128, 1000], DT)

            # fc6 partials (K-sharded): z6p[m8] = W6_slice.T @ pooled_slice.T
            for m8 in range(8):
                for nn in range(2):
                    ps = psum_pool.tile([128, 500], DT, tag="ps6")
                    for kt in range(KT6):
                        nc.tensor.matmul(ps[:], w6_sb[:, kt, m8, :],
                                         xk_sb[:, kt, nn * 500:(nn + 1) * 500],
                                         start=(kt == 0), stop=(kt == KT6 - 1))
                    zt = work_pool.tile([128, 500], DT, tag="z6")
                    nc.vector.tensor_copy(zt[:], ps[:])
                    nc.sync.dma_start(z6p[m8, :, nn * 500:(nn + 1) * 500], zt[:])

            nc.gpsimd.collective_compute(
                "AllReduce", mybir.AluOpType.add, replica_groups=groups,
                ins=[z6p.opt()], outs=[z6r.opt()])

            # x7 = relu(z6 + b6), all 8 m-tiles kept in SBUF
            x7_sb = big_pool.tile([128, 8, 1000], DT, tag="x78")
            for m8 in range(8):
                zin = work_pool.tile([128, 1000], DT, tag="zin")
                nc.sync.dma_start(zin[:], z6r[m8])
                nc.scalar.activation(x7_sb[:, m8], zin[:],
                                     mybir.ActivationFunctionType.Relu,
                                     bias=b6_sb[:, m8:m8 + 1])

            # fc7 N-shard: this core's 128 outputs over all 1000 rois
            x8_sb = big_pool.tile([128, 1000], DT)
            for nn in range(2):
                ps = psum_pool.tile([128, 500], DT, tag="ps7")
                for kt in range(8):
                    nc.tensor.matmul(ps[:], w7_sb[:, kt, :],
                                     x7_sb[:, kt, nn * 500:(nn + 1) * 500],
                                     start=(kt == 0), stop=(kt == 7))
                nc.scalar.activation(x8_sb[:, nn * 500:(nn + 1) * 500], ps[:],
                                     mybir.ActivationFunctionType.Relu,
                                     bias=b7_sb[:])
            nc.sync.dma_start(x8p[:], x8_sb[:])

            nc.gpsimd.collective_compute(
                "AllGather", mybir.AluOpType.bypass, replica_groups=groups,
                ins=[x8p.opt()], outs=[x8g.opt()])

            # heads N-shard: 51 output cols per core
            x8f_sb = big_pool.tile([128, 8, 1000], DT, tag="x78")
            for kt in range(8):
                nc.sync.dma_start(x8f_sb[:, kt], x8g[kt])
            oh_sb = work_pool.tile([51, 1000], DT, tag="oh")
            for nn in range(2):
                ps = psum_pool.tile([51, 500], DT, tag="psh")
                for kt in range(8):
                    nc.tensor.matmul(ps[:], wh_sb[:, kt, :],
                                     x8f_sb[:, kt, nn * 500:(nn + 1) * 500],
                                     start=(kt == 0), stop=(kt == 7))
                nc.vector.tensor_add(oh_sb[:, nn * 500:(nn + 1) * 500], ps[:],
                                     bh_sb.to_broadcast([51, 500]))
            nc.sync.dma_start(oh[:], oh_sb[:])
    nc.compile()
    return nc


def fc_in_maps(pooled, fc6_w, fc6_b, fc7_w, fc7_b, cls_w, cls_b, bbox_w, bbox_b):
    K6 = KT6 * 128 * NCORES
    pooledT = np.zeros((K6, 1000), F32)
    pooledT[:12544] = pooled.T
    w6T = np.zeros((K6, 1024), F32)
    w6T[:12544] = fc6_w.T
    wcb = np.zeros((408, 1024), F32)
    wcb[:81] = cls_w
    wcb[81:405] = bbox_w
    bcb = np.zeros((408,), F32)
    bcb[:81] = cls_b
    bcb[81:405] = bbox_b
    b6_arr = fc6_b.reshape(8, 128, 1).astype(F32)
    in_maps = []
    for c in range(NCORES):
        ks = slice(c * KT6 * 128, (c + 1) * KT6 * 128)
        m = dict(
            xk=np.ascontiguousarray(pooledT[ks].reshape(KT6, 128, 1000)),
            w6k=np.ascontiguousarray(w6T[ks].reshape(KT6, 128, 8, 128)),
            b6=b6_arr,
            w7k=np.ascontiguousarray(fc7_w[c * 128:(c + 1) * 128, :].T.reshape(8, 128, 128)),
            b7=fc7_b[c * 128:(c + 1) * 128].reshape(128, 1).astype(F32),
            whk=np.ascontiguousarray(wcb[c * 51:(c + 1) * 51].T.reshape(8, 128, 51)),
            bh=bcb[c * 51:(c + 1) * 51].reshape(51, 1),
        )
        in_maps.append(m)
    return in_maps


# ------------------------------------------------------------------- kernel
LAST_TIMES = {}


def kernel(p2, p3, p4, p5, p6, rpn_conv_w, rpn_conv_b, rpn_cls_w, rpn_cls_b,
           rpn_bbox_w, rpn_bbox_b, fc6_w, fc6_b, fc7_w, fc7_b,
           cls_w, cls_b, bbox_w, bbox_b):
    import time
    feats = [np.asarray(x, F32) for x in (p2, p3, p4, p5, p6)]

    if "rpn" not in _prog_cache:
        _prog_cache["rpn"] = build_rpn_program()
    if "fc" not in _prog_cache:
        _prog_cache["fc"] = build_fc_program()

    t0 = time.time()
    maps1 = rpn_in_maps(feats, np.asarray(rpn_conv_w, F32), np.asarray(rpn_cls_w, F32),
                        np.asarray(rpn_bbox_w, F32), np.asarray(rpn_conv_b, F32),
                        np.asarray(rpn_cls_b, F32), np.asarray(rpn_bbox_b, F32))
    t1 = time.time()
    res1 = run_bass_kernel_spmd(_prog_cache["rpn"], maps1, core_ids=list(range(NCORES)))
    t2 = time.time()
    cls_maps, bbox_maps = rpn_assemble(res1.results)
    props = rpn_proposals(cls_maps, bbox_maps)
    t3 = time.time()
    pooled = pooled_features([f[0] for f in feats[:4]], props)
    t4 = time.time()
    maps2 = fc_in_maps(pooled, np.asarray(fc6_w, F32), np.asarray(fc6_b, F32),
                       np.asarray(fc7_w, F32), np.asarray(fc7_b, F32),
                       np.asarray(cls_w, F32), np.asarray(cls_b, F32),
                       np.asarray(bbox_w, F32), np.asarray(bbox_b, F32))
    t5 = time.time()
    res2 = run_bass_kernel_spmd(_prog_cache["fc"], maps2, core_ids=list(range(NCORES)))
    t6 = time.time()
    ohs = np.stack([res2.results[c]["oh"] for c in range(NCORES)])  # [8,51,1000]
    outT = ohs.reshape(408, 1000)
    class_logits = np.ascontiguousarray(outT[:81].T)
    box_regression = np.ascontiguousarray(outT[81:405].T)
    LAST_TIMES.update(prep1=t1 - t0, call1=t2 - t1, host_props=t3 - t2,
                      roi_align=t4 - t3, prep2=t5 - t4, call2=t6 - t5)
    return class_logits, box_regression, props


# revision 5
# speedup vs baseline: 1.1186x; 1.0628x over previous
"""Faster R-CNN detection head (RPN + NMS + ROI-align + box head) on 8 trn2 cores.

Split:
  device call 1: RPN 3x3 conv + ReLU + 1x1 cls/bbox heads over 5 FPN levels
                 (rows sharded across 8 cores; conv = 9 shifted matmuls
                 accumulated in PSUM).
  host:          per-level top-k, box decode, greedy NMS, FPN top-k,
                 ROI-align bilinear pooling (numpy fp32, exact formula match).
  device call 2: fc6 (K-sharded over 8 cores) + AllReduce + fc7 (N-sharded)
                 + AllGather + cls/bbox heads (N-sharded).
"""
import numpy as np

import concourse.bass as bass
import concourse.mybir as mybir
import concourse.tile as tile
from concourse import bacc
from concourse.bass_utils import run_bass_kernel_spmd
import concourse.mybir as _mybir
import jax
from jax.experimental.shard_map import shard_map
from jax.sharding import Mesh, PartitionSpec
from concourse.bass2jax import _bass_exec_p, partition_id_tensor, install_neuronx_cc_hook


class SpmdRunner:
    """Build-once jitted SPMD executor for a compiled Bass program.

    Caches the jitted shard_map callable and device-resident input arrays for
    inputs marked static, so warm calls only stage the dynamic tensors.
    """

    def __init__(self, nc, n_cores=8):
        install_neuronx_cc_hook()
        self.nc = nc
        self.n_cores = n_cores
        in_names, out_names, out_avals, zero_outs = [], [], [], []
        partition_name = nc.partition_id_tensor.name if nc.partition_id_tensor else None
        for alloc in nc.m.functions[0].allocations:
            if not isinstance(alloc, mybir.MemoryLocationSet):
                continue
            name = alloc.memorylocations[0].name
            if alloc.kind == "ExternalInput":
                if name != partition_name:
                    in_names.append(name)
            elif alloc.kind == "ExternalOutput":
                out_names.append(name)
                shape = tuple(alloc.tensor_shape)
                dtype = mybir.dt.np(alloc.dtype)
                out_avals.append(jax.core.ShapedArray(shape, dtype))
                zero_outs.append(np.zeros((n_cores * shape[0], *shape[1:]), dtype))
        self.in_names = list(in_names)
        self.out_names = out_names
        self.out_avals = out_avals
        self.zero_outs = zero_outs
        n_params = len(in_names)
        n_outs = len(out_names)
        all_names = in_names + out_names
        if partition_name is not None:
            all_names.append(partition_name)

        def _body(*args):
            operands = list(args)
            if partition_name is not None:
                operands.append(partition_id_tensor())
            outs = _bass_exec_p.bind(
                *operands,
                out_avals=tuple(out_avals),
                in_names=tuple(all_names),
                out_names=tuple(out_names),
                lowering_input_output_aliases=(),
                sim_require_finite=True,
                sim_require_nnan=True,
                nc=nc,
            )
            return tuple(outs)

        devices = jax.devices()[:n_cores]
        self.mesh = Mesh(np.asarray(devices), ("core",))
        in_specs = (PartitionSpec("core"),) * (n_params + n_outs)
        out_specs = (PartitionSpec("core"),) * n_outs
        self.sharded = jax.jit(
            shard_map(_body, mesh=self.mesh, in_specs=in_specs,
                      out_specs=out_specs, check_rep=False),
            donate_argnums=tuple(range(n_params, n_params + n_outs)),
            keep_unused=True,
        )
        self._static = {}

    def put_static(self, name, per_core_or_shared):
        """Pre-stage an input on the devices; pass np array (shared across
        cores) or list of per-core np arrays."""
        v = per_core_or_shared
        if isinstance(v, list):
            cat = np.concatenate([np.asarray(x) for x in v], axis=0)
        else:
            a = np.asarray(v)
            cat = np.broadcast_to(a[None], (self.n_cores, *a.shape)).reshape(
                self.n_cores * a.shape[0], *a.shape[1:])
        sharding = jax.sharding.NamedSharding(self.mesh, PartitionSpec("core"))
        self._static[name] = jax.device_put(np.ascontiguousarray(cat), sharding)

    def __call__(self, dyn_in_maps):
        """dyn_in_maps: list per core of dicts for non-static inputs, or a
        single dict whose values are lists (per-core) / arrays (shared)."""
        sharding = jax.sharding.NamedSharding(self.mesh, PartitionSpec("core"))
        args = []
        for name in self.in_names:
            if name in self._static:
                args.append(self._static[name])
                continue
            v = dyn_in_maps[name]
            if isinstance(v, list):
                cat = np.concatenate([np.asarray(x) for x in v], axis=0)
            else:
                a = np.asarray(v)
                cat = np.broadcast_to(a[None], (self.n_cores, *a.shape)).reshape(
                    self.n_cores * a.shape[0], *a.shape[1:])
            args.append(jax.device_put(np.ascontiguousarray(cat), sharding))
        args.extend(self.zero_outs)
        out_arrs = self.sharded(*args)
        res = []
        for c in range(self.n_cores):
            res.append({
                name: np.asarray(out_arrs[i]).reshape(
                    self.n_cores, *self.out_avals[i].shape)[c]
                for i, name in enumerate(self.out_names)})
        return res

F32 = np.float32
DT = mybir.dt.float32
NCORES = 8

IMG_H, IMG_W = 800, 1280
STRIDES = (4, 8, 16, 32, 64)
SIZES = (32, 64, 128, 256, 512)
FEAT_HW = ((200, 320), (100, 160), (50, 80), (25, 40), (13, 20))
A = 3
PRE_NMS, POST_NMS, FPN_POST = 1000, 1000, 1000
NMS_TH = 0.7
POOL, SR = 7, 2
C = 256

# per-level sharding geometry: (H, W, Hc=rows/core, r=rows/group, G=groups/core)
GEOM = []
for (H, W) in FEAT_HW:
    Hc = -(-H // NCORES)          # ceil
    r = max(1, 512 // W)          # rows per PSUM group (N = r*W <= 512)
    r = min(r, Hc)
    G = -(-Hc // r)
    GEOM.append((H, W, Hc, r, G))
# -> p2:(200,320,25,1,25) p3:(100,160,13,3,5) p4:(50,80,7,6,2) p5:(25,40,4,4,1) p6:(13,20,2,2,1)

LEVELS = range(5)

_prog_cache = {}


# ---------------------------------------------------------------- call 1: RPN
def build_rpn_program():
    nc = bacc.Bacc(None, target_bir_lowering=False)
    xs, outs = [], []
    for l in LEVELS:
        H, W, Hc, r, G = GEOM[l]
        slab = G * r + 2
        xs.append(nc.declare_dram_parameter(f"x{l}", [2, 128, slab, W + 2], DT, isOutput=False))
    wconv = nc.declare_dram_parameter("wconv", [128, 9, 2, 256], DT, isOutput=False)
    whead = nc.declare_dram_parameter("whead", [128, 2, 15], DT, isOutput=False)
    cb = nc.declare_dram_parameter("cb", [128, 2], DT, isOutput=False)
    hb = nc.declare_dram_parameter("hb", [15, 1], DT, isOutput=False)
    for l in LEVELS:
        H, W, Hc, r, G = GEOM[l]
        outs.append(nc.declare_dram_parameter(f"o{l}", [G, 15, r * W], DT, isOutput=True))

    with tile.TileContext(nc) as tc:
        with (
            tc.tile_pool(name="const", bufs=1) as const_pool,
            tc.tile_pool(name="slabs", bufs=1) as slab_pool,
            tc.tile_pool(name="t", bufs=4) as t_pool,
            tc.tile_pool(name="ob", bufs=4) as out_pool,
            tc.tile_pool(name="ps", bufs=4, space="PSUM") as psum_pool,
            tc.tile_pool(name="ph", bufs=2, space="PSUM") as psum_head_pool,
        ):
            wc_sb = const_pool.tile([128, 9, 2, 256], DT)
            nc.sync.dma_start(wc_sb[:], wconv[:])
            wh_sb = const_pool.tile([128, 2, 15], DT)
            nc.sync.dma_start(wh_sb[:], whead[:])
            cb_sb = const_pool.tile([128, 2], DT)
            nc.sync.dma_start(cb_sb[:], cb[:])
            hb_sb = const_pool.tile([15, 1], DT)
            nc.sync.dma_start(hb_sb[:], hb[:])

            slab_sb = {}
            # issue loads small->large so small levels can start while p2 streams
            for l in [4, 3, 2, 1, 0]:
                H, W, Hc, r, G = GEOM[l]
                slab = G * r + 2
                s_sb = slab_pool.tile([128, 2, slab, W + 2], DT, tag=f"slab{l}")
                for kt in range(2):
                    nc.sync.dma_start(s_sb[:, kt], xs[l][kt])
                slab_sb[l] = s_sb

            for l in [4, 3, 2, 1, 0]:
                H, W, Hc, r, G = GEOM[l]
                N = r * W
                for g in range(G):
                    t_halves = []
                    for half in range(2):
                        ps = psum_pool.tile([128, N], DT, tag="ps")
                        for s in range(9):
                            di, dj = s // 3, s % 3
                            nc.tensor.matmul(
                                ps[:],
                                wc_sb[:, s, 0, half * 128:(half + 1) * 128],
                                slab_sb[l][:, 0, g * r + di:g * r + di + r, dj:dj + W],
                                start=(s == 0), stop=False,
                            )
                            nc.tensor.matmul(
                                ps[:],
                                wc_sb[:, s, 1, half * 128:(half + 1) * 128],
                                slab_sb[l][:, 1, g * r + di:g * r + di + r, dj:dj + W],
                                start=False, stop=(s == 8),
                            )
                        tt = t_pool.tile([128, N], DT, tag="t")
                        nc.scalar.activation(tt[:], ps[:],
                                             mybir.ActivationFunctionType.Relu,
                                             bias=cb_sb[:, half:half + 1])
                        t_halves.append(tt)
                    ph = psum_head_pool.tile([15, N], DT, tag="ph")
                    for kt in range(2):
                        nc.tensor.matmul(ph[:], wh_sb[:, kt, :], t_halves[kt][:],
                                         start=(kt == 0), stop=(kt == 1))
                    ob = out_pool.tile([15, N], DT, tag="ob")
                    nc.vector.tensor_add(ob[:], ph[:], hb_sb.to_broadcast([15, N]))
                    nc.sync.dma_start(outs[l][g], ob[:])
    nc.compile()
    return nc


def rpn_in_maps(feats, rpn_conv_w, rpn_cls_w, rpn_bbox_w, rpn_conv_b, rpn_cls_b, rpn_bbox_b):
    w_r = rpn_conv_w.reshape(256, 2, 128, 3, 3)
    wconv_arr = np.ascontiguousarray(
        np.transpose(w_r, (2, 3, 4, 1, 0)).reshape(128, 9, 2, 256), F32)
    hw = np.concatenate([rpn_cls_w[:, :, 0, 0], rpn_bbox_w[:, :, 0, 0]], 0)  # [15,256]
    whead_arr = np.ascontiguousarray(hw.reshape(15, 2, 128).transpose(2, 1, 0), F32)
    cb_arr = np.ascontiguousarray(rpn_conv_b.reshape(2, 128).T, F32)
    hb_arr = np.concatenate([rpn_cls_b, rpn_bbox_b])[:, None].astype(F32)

    slabs_per_core = [[] for _ in range(NCORES)]
    for l in LEVELS:
        H, W, Hc, r, G = GEOM[l]
        slab = G * r + 2
        Hbig = (NCORES - 1) * Hc + slab
        xp = np.zeros((2, 128, Hbig, W + 2), F32)
        xp[:, :, 1:H + 1, 1:W + 1] = feats[l][0].reshape(2, 128, H, W)
        for c in range(NCORES):
            slabs_per_core[c].append(
                np.ascontiguousarray(xp[:, :, c * Hc:c * Hc + slab]))
    in_maps = []
    for c in range(NCORES):
        m = {f"x{l}": slabs_per_core[c][l] for l in LEVELS}
        m.update(wconv=wconv_arr, whead=whead_arr, cb=cb_arr, hb=hb_arr)
        in_maps.append(m)
    return in_maps


def rpn_assemble(results):
    """per-core outputs -> full cls [A,H,W] and bbox [12,H,W] maps per level"""
    cls_maps, bbox_maps = [], []
    for l in LEVELS:
        H, W, Hc, r, G = GEOM[l]
        full = np.empty((15, H, W), F32)
        for c in range(NCORES):
            o = results[c][f"o{l}"].reshape(G, 15, r, W).transpose(1, 0, 2, 3).reshape(15, G * r, W)
            lo = c * Hc
            hi = min(H, lo + Hc)
            if lo >= H:
                continue
            full[:, lo:hi] = o[:, :hi - lo]
        cls_maps.append(full[:3])
        bbox_maps.append(full[3:])
    return cls_maps, bbox_maps


# ------------------------------------------------------------- host pipeline
def make_anchors(size, stride, H, W):
    ratios = np.array([0.5, 1.0, 2.0], F32)
    ws = np.sqrt(F32(size * size) / ratios)
    hs = ws * ratios
    base = np.stack([-ws / 2, -hs / 2, ws / 2, hs / 2], axis=1).astype(F32)
    X, Y = np.meshgrid(np.arange(W, dtype=F32) * F32(stride),
                       np.arange(H, dtype=F32) * F32(stride))
    shifts = np.stack([X, Y, X, Y], axis=-1).reshape(-1, 4).astype(F32)
    return (shifts[:, None, :] + base[None, :, :]).reshape(-1, 4)


def decode(rel, boxes):
    w = boxes[:, 2] - boxes[:, 0] + F32(1.0)
    h = boxes[:, 3] - boxes[:, 1] + F32(1.0)
    cx = boxes[:, 0] + F32(0.5) * w
    cy = boxes[:, 1] + F32(0.5) * h
    dx, dy, dw, dh = rel[:, 0], rel[:, 1], rel[:, 2], rel[:, 3]
    bbox_clip = np.log(F32(1000.0 / 16)).astype(F32)
    dw = np.minimum(dw, bbox_clip)
    dh = np.minimum(dh, bbox_clip)
    pcx = dx * w + cx
    pcy = dy * h + cy
    pw = np.exp(dw) * w
    ph = np.exp(dh) * h
    return np.stack([pcx - F32(0.5) * pw, pcy - F32(0.5) * ph,
                     pcx + F32(0.5) * pw - 1, pcy + F32(0.5) * ph - 1], axis=1)


def clip_boxes(b):
    return np.stack([np.clip(b[:, 0], 0, IMG_W - 1), np.clip(b[:, 1], 0, IMG_H - 1),
                     np.clip(b[:, 2], 0, IMG_W - 1), np.clip(b[:, 3], 0, IMG_H - 1)], axis=1)


def box_iou_one(box, boxes):
    lt = np.maximum(box[:2], boxes[:, :2])
    rb = np.minimum(box[2:], boxes[:, 2:])
    wh = np.clip(rb - lt + F32(1.0), 0, None)
    inter = wh[:, 0] * wh[:, 1]
    a1 = (box[2] - box[0] + F32(1.0)) * (box[3] - box[1] + F32(1.0))
    a2 = (boxes[:, 2] - boxes[:, 0] + F32(1.0)) * (boxes[:, 3] - boxes[:, 1] + F32(1.0))
    return inter / (a1 + a2 - inter)


def nms(boxes, scores, iou_th, max_out):
    order = np.argsort(-scores, kind="stable")
    b = boxes[order]
    n = b.shape[0]
    sup = np.zeros((n,), bool)
    keep = np.full((max_out,), -1, np.int32)
    cnt = 0
    for _ in range(max_out):
        nsup = ~sup
        if not nsup.any():
            break
        idx = int(np.argmax(nsup))
        ious = box_iou_one(b[idx], b)
        sup |= ious > F32(iou_th)
        keep[cnt] = order[idx]
        cnt += 1
    return keep


def sigmoid(x):
    out = np.empty_like(x)
    pos = x >= 0
    out[pos] = F32(1.0) / (F32(1.0) + np.exp(-x[pos]))
    ex = np.exp(x[~pos])
    out[~pos] = ex / (F32(1.0) + ex)
    return out


def level_proposals(cls_map, bbox_map, anchors):
    H, W = cls_map.shape[1], cls_map.shape[2]
    logits = np.transpose(cls_map, (1, 2, 0)).reshape(-1)
    deltas = np.transpose(bbox_map.reshape(A, 4, H, W), (2, 3, 0, 1)).reshape(-1, 4)
    k = min(PRE_NMS, logits.shape[0])
    order = np.argsort(-logits, kind="stable")[:k]
    top_logits = logits[order]
    boxes = clip_boxes(decode(deltas[order], anchors[order]))
    keep = nms(boxes, top_logits, NMS_TH, POST_NMS)
    valid = keep >= 0
    ki = np.maximum(keep, 0)
    out_boxes = np.where(valid[:, None], boxes[ki], F32(0.0)).astype(F32)
    out_scores = np.where(valid, sigmoid(top_logits[ki]), F32(-1.0)).astype(F32)
    return out_boxes, out_scores


def rpn_proposals(cls_maps, bbox_maps):
    all_boxes, all_scores = [], []
    for l, (stride, size, (H, W)) in enumerate(zip(STRIDES, SIZES, FEAT_HW)):
        anchors = make_anchors(size, stride, H, W)
        b, s = level_proposals(cls_maps[l], bbox_maps[l], anchors)
        all_boxes.append(b)
        all_scores.append(s)
    boxes = np.concatenate(all_boxes, 0)
    scores = np.concatenate(all_scores, 0)
    top = np.argsort(-scores, kind="stable")[:FPN_POST]
    return boxes[top]


def roi_align(featT, H, W, rois, scale):
    # featT: [H*W, C] row-major (channel-last) for contiguous row gathers
    N = rois.shape[0]
    Cc = featT.shape[1]
    S = POOL * SR
    x1 = rois[:, 0] * F32(scale)
    y1 = rois[:, 1] * F32(scale)
    rw = np.maximum(rois[:, 2] * F32(scale) - x1, F32(1.0))
    rh = np.maximum(rois[:, 3] * F32(scale) - y1, F32(1.0))
    g = ((np.arange(S, dtype=F32) + F32(0.5)) / F32(SR)).astype(F32)
    xs = x1[:, None] + g[None, :] * (rw / F32(POOL))[:, None]
    ys = y1[:, None] + g[None, :] * (rh / F32(POOL))[:, None]
    y = np.broadcast_to(ys[:, :, None], (N, S, S))
    x = np.broadcast_to(xs[:, None, :], (N, S, S))
    mask = (y > -1.0) & (y < H) & (x > -1.0) & (x < W)
    y = np.clip(y, F32(0.0), F32(H - 1))
    x = np.clip(x, F32(0.0), F32(W - 1))
    y0f = np.floor(y)
    x0f = np.floor(x)
    y0 = y0f.astype(np.int32)
    x0 = x0f.astype(np.int32)
    y1i = np.minimum(y0 + 1, H - 1)
    x1i = np.minimum(x0 + 1, W - 1)
    ly = (y - y0f).astype(F32)
    lx = (x - x0f).astype(F32)
    hy = F32(1) - ly
    hx = F32(1) - lx
    m = mask.astype(F32)
    w00 = (hy * hx * m).reshape(-1, 1)
    w01 = (hy * lx * m).reshape(-1, 1)
    w10 = (ly * hx * m).reshape(-1, 1)
    w11 = (ly * lx * m).reshape(-1, 1)
    i00 = (y0 * W + x0).ravel()
    i01 = (y0 * W + x1i).ravel()
    i10 = (y1i * W + x0).ravel()
    i11 = (y1i * W + x1i).ravel()
    val = w00 * featT[i00]
    val += w01 * featT[i01]
    val += w10 * featT[i10]
    val += w11 * featT[i11]                     # [N*S*S, C]
    val = val.reshape(N, POOL, SR, POOL, SR, Cc).mean(axis=(2, 4), dtype=F32)
    return np.transpose(val, (0, 3, 1, 2))      # [N, C, 7, 7]


def pooled_features(feats, proposals):
    area = np.maximum((proposals[:, 2] - proposals[:, 0]) *
                      (proposals[:, 3] - proposals[:, 1]), F32(1e-6))
    lvl = np.floor(F32(4) + np.log2(np.sqrt(area) / F32(224.0) + F32(1e-6)))
    lvl = np.clip(lvl, 2, 5).astype(np.int32) - 2
    pooled = np.zeros((proposals.shape[0], C, POOL, POOL), F32)
    for l in range(4):
        sel = np.nonzero(lvl == l)[0]
        if sel.size == 0:
            continue
        H, W = FEAT_HW[l]
        featT = np.ascontiguousarray(feats[l].reshape(C, H * W).T)
        pooled[sel] = roi_align(featT, H, W, proposals[sel], 1.0 / STRIDES[l])
    return pooled.reshape(proposals.shape[0], -1)


# ---------------------------------------------------------------- call 2: FC
KT6 = 13          # k-tiles of 128 per core for fc6 (13*128*8 = 13312 >= 12544)


def build_fc_program():
    nc = bacc.Bacc(None, target_bir_lowering=False)
    xk = nc.declare_dram_parameter("xk", [KT6, 128, 1000], DT, isOutput=False)
    w6k = nc.declare_dram_parameter("w6k", [KT6, 128, 8, 128], DT, isOutput=False)
    b6 = nc.declare_dram_parameter("b6", [8, 128, 1], DT, isOutput=False)
    w7k = nc.declare_dram_parameter("w7k", [8, 128, 128], DT, isOutput=False)
    b7 = nc.declare_dram_parameter("b7", [128, 1], DT, isOutput=False)
    whk = nc.declare_dram_parameter("whk", [8, 128, 51], DT, isOutput=False)
    bh = nc.declare_dram_parameter("bh", [51, 1], DT, isOutput=False)
    oh = nc.declare_dram_parameter("oh", [51, 1000], DT, isOutput=True)

    groups = [list(range(NCORES))]
    with tile.TileContext(nc) as tc:
        with (
            tc.tile_pool(name="big", bufs=1) as big_pool,
            tc.tile_pool(name="work", bufs=2) as work_pool,
            tc.tile_pool(name="ps", bufs=2, space="PSUM") as psum_pool,
            tc.tile_pool(name="dram", bufs=1, space="DRAM") as dram_pool,
        ):
            xk_sb = big_pool.tile([128, KT6, 1000], DT)
            for kt in range(KT6):
                nc.sync.dma_start(xk_sb[:, kt], xk[kt])
            w6_sb = big_pool.tile([128, KT6, 8, 128], DT)
            for kt in range(KT6):
                nc.sync.dma_start(w6_sb[:, kt], w6k[kt])
            b6_sb = big_pool.tile([128, 8], DT)
            for m8 in range(8):
                nc.sync.dma_start(b6_sb[:, m8:m8 + 1], b6[m8])
            w7_sb = big_pool.tile([128, 8, 128], DT)
            for kt in range(8):
                nc.sync.dma_start(w7_sb[:, kt], w7k[kt])
            b7_sb = big_pool.tile([128, 1], DT)
            nc.sync.dma_start(b7_sb[:], b7[:])
            wh_sb = big_pool.tile([128, 8, 51], DT)
            for kt in range(8):
                nc.sync.dma_start(wh_sb[:, kt], whk[kt])
            bh_sb = big_pool.tile([51, 1], DT)
            nc.sync.dma_start(bh_sb[:], bh[:])

            z6p = dram_pool.tile([8, 128, 1000], DT)
            z6r = dram_pool.tile([8, 128, 1000], DT)
            x8p = dram_pool.tile([128, 1000], DT)
            x8g = dram_pool.tile([8, 128, 1000], DT)

            # fc6 partials (K-sharded): z6p[m8] = W6_slice.T @ pooled_slice.T
            for m8 in range(8):
                for nn in range(2):
                    ps = psum_pool.tile([128, 500], DT, tag="ps6")
                    for kt in range(KT6):
                        nc.tensor.matmul(ps[:], w6_sb[:, kt, m8, :],
                                         xk_sb[:, kt, nn * 500:(nn + 1) * 500],
                                         start=(kt == 0), stop=(kt == KT6 - 1))
                    zt = work_pool.tile([128, 500], DT, tag="z6")
                    nc.vector.tensor_copy(zt[:], ps[:])
                    nc.sync.dma_start(z6p[m8, :, nn * 500:(nn + 1) * 500], zt[:])

            nc.gpsimd.collective_compute(
                "AllReduce", mybir.AluOpType.add, replica_groups=groups,
                ins=[z6p.opt()], outs=[z6r.opt()])

            # x7 = relu(z6 + b6), all 8 m-tiles kept in SBUF
            x7_sb = big_pool.tile([128, 8, 1000], DT, tag="x78")
            for m8 in range(8):
                zin = work_pool.tile([128, 1000], DT, tag="zin")
                nc.sync.dma_start(zin[:], z6r[m8])
                nc.scalar.activation(x7_sb[:, m8], zin[:],
                                     mybir.ActivationFunctionType.Relu,
                                     bias=b6_sb[:, m8:m8 + 1])

            # fc7 N-shard: this core's 128 outputs over all 1000 rois
            x8_sb = big_pool.tile([128, 1000], DT)
            for nn in range(2):
                ps = psum_pool.tile([128, 500], DT, tag="ps7")
                for kt in range(8):
                    nc.tensor.matmul(ps[:], w7_sb[:, kt, :],
                                     x7_sb[:, kt, nn * 500:(nn + 1) * 500],
                                     start=(kt == 0), stop=(kt == 7))
                nc.scalar.activation(x8_sb[:, nn * 500:(nn + 1) * 500], ps[:],
                                     mybir.ActivationFunctionType.Relu,
                                     bias=b7_sb[:])
            nc.sync.dma_start(x8p[:], x8_sb[:])

            nc.gpsimd.collective_compute(
                "AllGather", mybir.AluOpType.bypass, replica_groups=groups,
                ins=[x8p.opt()], outs=[x8g.opt()])

            # heads N-shard: 51 output cols per core
            x8f_sb = big_pool.tile([128, 8, 1000], DT, tag="x78")
            for kt in range(8):
                nc.sync.dma_start(x8f_sb[:, kt], x8g[kt])
            oh_sb = work_pool.tile([51, 1000], DT, tag="oh")
            for nn in range(2):
                ps = psum_pool.tile([51, 500], DT, tag="psh")
                for kt in range(8):
                    nc.tensor.matmul(ps[:], wh_sb[:, kt, :],
                                     x8f_sb[:, kt, nn * 500:(nn + 1) * 500],
                                     start=(kt == 0), stop=(kt == 7))
                nc.vector.tensor_add(oh_sb[:, nn * 500:(nn + 1) * 500], ps[:],
                                     bh_sb.to_broadcast([51, 500]))
            nc.sync.dma_start(oh[:], oh_sb[:])
    nc.compile()
    return nc


def fc_in_maps(pooled, fc6_w, fc6_b, fc7_w, fc7_b, cls_w, cls_b, bbox_w, bbox_b):
    K6 = KT6 * 128 * NCORES
    pooledT = np.zeros((K6, 1000), F32)
    pooledT[:12544] = pooled.T
    w6T = np.zeros((K6, 1024), F32)
    w6T[:12544] = fc6_w.T
    wcb = np.zeros((408, 1024), F32)
    wcb[:81] = cls_w
    wcb[81:405] = bbox_w
    bcb = np.zeros((408,), F32)
    bcb[:81] = cls_b
    bcb[81:405] = bbox_b
    b6_arr = fc6_b.reshape(8, 128, 1).astype(F32)
    in_maps = []
    for c in range(NCORES):
        ks = slice(c * KT6 * 128, (c + 1) * KT6 * 128)
        m = dict(
            xk=np.ascontiguousarray(pooledT[ks].reshape(KT6, 128, 1000)),
            w6k=np.ascontiguousarray(w6T[ks].reshape(KT6, 128, 8, 128)),
            b6=b6_arr,
            w7k=np.ascontiguousarray(fc7_w[c * 128:(c + 1) * 128, :].T.reshape(8, 128, 128)),
            b7=fc7_b[c * 128:(c + 1) * 128].reshape(128, 1).astype(F32),
            whk=np.ascontiguousarray(wcb[c * 51:(c + 1) * 51].T.reshape(8, 128, 51)),
            bh=bcb[c * 51:(c + 1) * 51].reshape(51, 1),
        )
        in_maps.append(m)
    return in_maps


# ------------------------------------------------------------------- kernel
LAST_TIMES = {}


def kernel(p2, p3, p4, p5, p6, rpn_conv_w, rpn_conv_b, rpn_cls_w, rpn_cls_b,
           rpn_bbox_w, rpn_bbox_b, fc6_w, fc6_b, fc7_w, fc7_b,
           cls_w, cls_b, bbox_w, bbox_b):
    import time
    feats = [np.asarray(x, F32) for x in (p2, p3, p4, p5, p6)]

    if "rpn" not in _prog_cache:
        _prog_cache["rpn"] = SpmdRunner(build_rpn_program())
    if "fc" not in _prog_cache:
        _prog_cache["fc"] = SpmdRunner(build_fc_program())

    t0 = time.time()
    maps1 = rpn_in_maps(feats, np.asarray(rpn_conv_w, F32), np.asarray(rpn_cls_w, F32),
                        np.asarray(rpn_bbox_w, F32), np.asarray(rpn_conv_b, F32),
                        np.asarray(rpn_cls_b, F32), np.asarray(rpn_bbox_b, F32))
    t1 = time.time()
    dyn1 = {k: [m[k] for m in maps1] for k in maps1[0]}
    res1 = _prog_cache["rpn"](dyn1)
    t2 = time.time()
    cls_maps, bbox_maps = rpn_assemble(res1)
    props = rpn_proposals(cls_maps, bbox_maps)
    t3 = time.time()
    pooled = pooled_features([f[0] for f in feats[:4]], props)
    t4 = time.time()
    maps2 = fc_in_maps(pooled, np.asarray(fc6_w, F32), np.asarray(fc6_b, F32),
                       np.asarray(fc7_w, F32), np.asarray(fc7_b, F32),
                       np.asarray(cls_w, F32), np.asarray(cls_b, F32),
                       np.asarray(bbox_w, F32), np.asarray(bbox_b, F32))
    t5 = time.time()
    dyn2 = {k: [m[k] for m in maps2] for k in maps2[0]}
    res2 = _prog_cache["fc"](dyn2)
    t6 = time.time()
    ohs = np.stack([res2[c]["oh"] for c in range(NCORES)])  # [8,51,1000]
    outT = ohs.reshape(408, 1000)
    class_logits = np.ascontiguousarray(outT[:81].T)
    box_regression = np.ascontiguousarray(outT[81:405].T)
    LAST_TIMES.update(prep1=t1 - t0, call1=t2 - t1, host_props=t3 - t2,
                      roi_align=t4 - t3, prep2=t5 - t4, call2=t6 - t5)
    return class_logits, box_regression, props
